# revision 48
# baseline (speedup 1.0000x reference)
"""SAGAN-style attention block (B=16, C=64, 64x64) on 8 TRN2 NeuronCores.

Data-parallel over batch: each core processes 2 batch elements end-to-end.

Per-batch pipeline (channel-major [C, N] layout, N = 4096 queries,
M = 1024 pooled keys):
  1. pre = [w_theta; w_phi; w_g] @ x -> [96, N] (one fused 1x1 conv matmul;
     rows padded so theta/phi/g land on 32-aligned partition bases)
  2. 2x2 maxpool of the phi/g rows over the 64x64 grid -> phi [8, M], g [32, M]
  3. scoresT[m, n] = phi^T theta computed directly in [M, N] orientation
     (lhsT = phi tile, rhs = theta chunk; contraction K = 8)
  4. E = exp(scoresT)  (no max subtraction: |s| <~ 6, exp is safe in fp32);
     exp granularity [128, 1024] (two m-tiles per psum tile) to amortize the
     per-ACTIVATE overhead — the scalar engine is the kernel's bottleneck
  5. O = [g; ones]^T-augmented PV matmul: lhsT = [gT | 1] [m, 33] so row 32
     accumulates the softmax denominator for free
  6. out = (gamma*w_o) @ O[0:32] * recip(denom) + x, denominator applied
     per-column via a gpsimd partition-broadcast

Schedule: both batches' x loads are issued up front; batch 0's pre-conv psum
copies run on the otherwise-idle scalar engine; batch 1's whole preamble is
interleaved into batch 0's main loop; batch 1's first score pairs are emitted
ahead of batch 0's final PV phase so the activation engine never drains at
the batch boundary; the final chunk is split in half to shorten the serial
epilogue. All matmuls run as float32r (1 cycle/column at N>=256, ~1.6e-4
elementwise rounding), with fp32 PSUM accumulation; end-to-end relative
error vs the fp32 reference is ~1.4e-5.
"""

import functools
import sys

import numpy as np

sys.path.insert(0, "/opt/trn_rl_repo")

import concourse.bacc as bacc
import concourse.mybir as mybir
import concourse.tile as tile
from concourse.bass_utils import run_bass_kernel_spmd

B, C, H, W = 16, 64, 64, 64
N = H * W            # 4096 queries
M = N // 4           # 1024 keys after 2x2 maxpool
NCORES = 8
BPC = B // NCORES    # batches per core
DH = 8               # theta/phi channels (C // HEADS)
CG = 32              # g channels (C // 2)
NCHUNK = 512
NCH = N // NCHUNK    # 8 chunks per batch
MT = M // 128        # 8 m-tiles of 128

F32 = mybir.dt.float32
F32R = mybir.dt.float32r
EXP = mybir.ActivationFunctionType.Exp


def _emit(nc, tc, x2, x2r, wallT, woT, ident, y2):
    with (
        tc.tile_pool(name="const", bufs=1) as pconst,
        tc.tile_pool(name="big", bufs=2) as pbig,
        tc.tile_pool(name="scr", bufs=1) as pscr,
        tc.tile_pool(name="ep", bufs=8) as pe_pool,
        tc.tile_pool(name="sm", bufs=3) as psm,
        tc.tile_pool(name="xr", bufs=6) as pxr,
        tc.tile_pool(name="psc", bufs=3, space="PSUM") as psc,
        tc.tile_pool(name="ppv", bufs=1, space="PSUM") as ppv,
        tc.tile_pool(name="pcv", bufs=1, space="PSUM") as pcv,
    ):
        # --- replicated weights: DMA in fp32, round once to fp32r ---
        wallT_s = pconst.tile([C, 96], F32)
        woT_s = pconst.tile([CG, C], F32)
        ident_s = pconst.tile([32, 32], F32)
        wallT_r = pconst.tile([C, 96], F32R)
        woT_r = pconst.tile([CG, C], F32R)
        ident_r = pconst.tile([32, 32], F32R)
        ones_c = pconst.tile([128, MT], F32)
        ones_r = pconst.tile([1, 1], F32R)

        def emit_weights_head():
            # only wallT gates the pre-conv critical path
            nc.sync.dma_start(wallT_s[:], wallT.ap())
            nc.vector.tensor_copy(wallT_r[:], wallT_s[:])

        def emit_weights_rest():
            nc.sync.dma_start(woT_s[:], woT.ap())
            nc.sync.dma_start(ident_s[:], ident.ap())
            nc.vector.tensor_copy(woT_r[:], woT_s[:])
            nc.vector.tensor_copy(ident_r[:], ident_s[:])
            nc.gpsimd.memset(ones_c[:], 1.0)
            nc.vector.tensor_copy(ones_r[:], ones_c[0:1, 0:1])

        st = [{} for _ in range(BPC)]

        # All x loads for the fused conv are issued first: the cost of a DMA
        # queue slot is high and these gate the whole pre pipeline.
        XRC_PIECES = [(0, 1024), (1024, 1024), (2048, 1024), (3072, 1024)]

        def emit_xrc(b):
            st[b]["xrc"] = []
            for off, wdt in XRC_PIECES:
                xrc = pxr.tile([C, wdt], F32R, tag="xrc", name="xrc")
                nc.sync.dma_start(xrc[:], x2r.ap()[b, :, off:off + wdt])
                st[b]["xrc"].append((off, xrc))

        def emit_alloc(b):
            s_ = st[b]
            # fused conv output, rows padded for 32-aligned partition bases:
            # rows 0:8 theta, 32:40 phi, 64:96 g
            s_["pre"] = pbig.tile([96, N], F32R, tag="pre", name="pre_s")
            s_["t1p"] = pscr.tile([DH, 64, 32], F32, tag="t1phi", name="t1_phi")
            s_["t1g"] = pscr.tile([CG, 64, 32], F32, tag="t1g", name="t1_g")
            s_["phi"] = pbig.tile([DH, 32, 32], F32R, tag="phir", name="phi_r")
            s_["g"] = pbig.tile([CG, 32, 32], F32R, tag="gr", name="g_r")
            s_["gTa"] = pbig.tile([128, MT, 33], F32R, tag="gTa", name="gTa")
            nc.vector.tensor_copy(s_["gTa"][:, :, 32], ones_c[:])

        def emit_pre(b, lo, hi):
            s_ = st[b]
            for j in range(lo, hi):
                # ping-pong the pv/cv pools: during preambles they are idle
                # and the b1 pieces slot between main-loop uses with slack
                pool_j = ppv if j % 2 == 0 else pcv
                pp = pool_j.tile([96, NCHUNK], F32,
                                 tag="pv" if j % 2 == 0 else "cv")
                n0 = j * NCHUNK
                off, xrc = next(
                    (o, t) for o, t in s_["xrc"]
                    if o <= n0 < o + t.shape[1]
                )
                nc.tensor.matmul(
                    pp[:], wallT_r[:], xrc[:, n0 - off:n0 - off + NCHUNK],
                    start=True, stop=True,
                )
                dst = s_["pre"][:, j * NCHUNK:(j + 1) * NCHUNK]
                if b == 0:
                    # ACT is idle before the first exp and Copy shares every
                    # activation table with Exp (no table reload)
                    nc.scalar.copy(dst, pp[:])
                else:
                    # batch 1's copies overlap batch 0's exp-saturated window
                    nc.vector.tensor_copy(dst, pp[:])

        # 2x2 maxpool, split spatially so downstream work can start as soon
        # as the pre chunks feeding each piece are in. qn = number of
        # quarters (16 spatial rows each), qt = starting quarter.
        def emit_phi_piece(b, qt, qn):
            s_ = st[b]
            pool_in = s_["pre"][32:40, :].bitcast(F32).rearrange(
                "c (h w2 two) -> c h w2 two", two=2, w2=32
            )
            hs = slice(16 * qt, 16 * (qt + qn))
            nc.vector.tensor_max(
                s_["t1p"][:, hs, :], pool_in[:, hs, :, 0], pool_in[:, hs, :, 1]
            )
            t1v = s_["t1p"][:].rearrange("c (h2 two) w2 -> c h2 two w2", two=2)
            h2 = slice(8 * qt, 8 * (qt + qn))
            nc.vector.tensor_max(
                s_["phi"][:, h2, :], t1v[:, h2, 0, :], t1v[:, h2, 1, :]
            )

        def emit_g_half(b, hf):
            s_ = st[b]
            pool_in = s_["pre"][64:96, :].bitcast(F32).rearrange(
                "c (h w2 two) -> c h w2 two", two=2, w2=32
            )
            hs = slice(32 * hf, 32 * hf + 32)
            nc.vector.tensor_max(
                s_["t1g"][:, hs, :], pool_in[:, hs, :, 0], pool_in[:, hs, :, 1]
            )
            t1v = s_["t1g"][:].rearrange("c (h2 two) w2 -> c h2 two w2", two=2)
            h2 = slice(16 * hf, 16 * hf + 16)
            nc.vector.tensor_max(
                s_["g"][:, h2, :], t1v[:, h2, 0, :], t1v[:, h2, 1, :]
            )
            # transposes for this half's 4 m-tiles into the augmented lhsT
            g_flat = s_["g"][:].rearrange("c h w -> c (h w)")
            trp = pcv.tile([128, 128], F32R, tag="cv")
            for q in range(4):
                mi = 4 * hf + q
                nc.tensor.transpose(
                    trp[:, q * 32:(q + 1) * 32],
                    g_flat[:, mi * 128:(mi + 1) * 128],
                    ident_r[:],
                )
            nc.vector.tensor_copy(
                s_["gTa"][:, 4 * hf:4 * hf + 4, 0:32],
                trp[:].rearrange("p (four c) -> p four c", c=32),
            )

        def emit_xb(b):
            st[b]["xb"] = pbig.tile([C, N], F32, tag="xb", name="xb")
            nc.sync.dma_start(st[b]["xb"][:], x2.ap()[b])

        M_GROUPS = [(0, 1), (2, 3), (4, 5), (6, 7)]

        def emit_sc(b, n0, nw, groups):
            s_ = st[b]
            phi_r = s_["phi"][:].rearrange("c h w -> c (h w)")
            th = s_["pre"][0:8, n0:n0 + nw]  # theta chunk, f32r
            es = []
            for grp in groups:  # up to 3 m-tiles share one psum tile / exp
                scp_t = psc.tile([128, len(grp) * nw], F32, tag="sc")
                for h_, mi in enumerate(grp):
                    nc.tensor.matmul(
                        scp_t[:, h_ * nw:(h_ + 1) * nw],
                        phi_r[:, mi * 128:(mi + 1) * 128],
                        th,
                        start=True, stop=True,
                    )
                e_t = pe_pool.tile([128, len(grp) * nw], F32R, tag="E")
                nc.scalar.activation(e_t[:], scp_t[:], EXP)
                es.append(e_t)
            return es

        def emit_main_chunk(b, n0, nw=NCHUNK, tail=False, es=None):
            s_ = st[b]
            gTa = s_["gTa"]
            if es is None:
                es = emit_sc(b, n0, nw, M_GROUPS)

            pv = ppv.tile([33, nw], F32, tag="pv")
            for gi, grp in enumerate(M_GROUPS):
                for h_, mi in enumerate(grp):
                    nc.tensor.matmul(
                        pv[:],
                        gTa[:, mi, :],
                        es[gi][:, h_ * nw:(h_ + 1) * nw],
                        start=(mi == 0), stop=(mi == MT - 1),
                    )

            o_sb = psm.tile([CG, nw], F32R, tag="osb")
            nc.vector.tensor_copy(o_sb[:], pv[0:CG, :])
            cv = pcv.tile([C, nw], F32, tag="cv")
            nc.tensor.matmul(cv[:], woT_r[:], o_sb[:], start=True, stop=True)
            recip = psm.tile([1, nw], F32, tag="rc")
            nc.vector.reciprocal(recip[:], pv[CG:CG + 1, :])
            rb = psm.tile([C, nw], F32, tag="rb")
            nc.gpsimd.partition_broadcast(rb[:], recip[:])
            prod = psm.tile([C, nw], F32, tag="prod")
            nc.vector.tensor_mul(prod[:], cv[:], rb[:])
            out_sb = psm.tile([C, nw], F32, tag="out")
            if tail:
                # the kernel tail is this serial chain; DVE is faster than
                # gpsimd for the final adds
                nc.vector.tensor_add(out_sb[:], prod[:], s_["xb"][:, n0:n0 + nw])
            else:
                nc.gpsimd.tensor_add(out_sb[:], prod[:], s_["xb"][:, n0:n0 + nw])
            nc.sync.dma_start(y2.ap()[b, :, n0:n0 + nw], out_sb[:])

        # -------- schedule --------
        emit_weights_head()
        emit_xrc(0)
        emit_xrc(1)
        emit_weights_rest()
        emit_alloc(0)
        emit_pre(0, 0, 4)
        emit_phi_piece(0, 0, 2)
        emit_pre(0, 4, 8)
        emit_phi_piece(0, 2, 2)
        emit_g_half(0, 0)
        emit_g_half(0, 1)
        emit_xb(0)
        emit_alloc(1)
        # batch 1's preamble pieces ride inside batch 0's main loop: by the
        # time each piece is reached its DMA input has long landed, so the
        # PE/DVE never head-of-line block on it
        b1_pieces = {
            0: [lambda: emit_pre(1, 0, 2)],
            1: [lambda: emit_pre(1, 2, 3)],
            2: [lambda: emit_pre(1, 3, 4), lambda: emit_phi_piece(1, 0, 2)],
            3: [lambda: emit_pre(1, 4, 6), lambda: emit_g_half(1, 0)],
            4: [lambda: emit_pre(1, 6, 7)],
            5: [lambda: emit_pre(1, 7, 8), lambda: emit_phi_piece(1, 2, 2),
                lambda: emit_g_half(1, 1), lambda: emit_xb(1)],
        }
        for j in range(NCH - 1):
            emit_main_chunk(0, j * NCHUNK)
            for fn in b1_pieces.get(j, []):
                fn()
        # handoff: batch 1's first score pairs go ahead of batch 0's final
        # PV phase so the activation engine never drains at the boundary
        es_b1c0 = emit_sc(1, 0, NCHUNK, M_GROUPS[:2])
        emit_main_chunk(0, (NCH - 1) * NCHUNK)
        es_b1c0 += emit_sc(1, 0, NCHUNK, M_GROUPS[2:])
        emit_main_chunk(1, 0, es=es_b1c0)
        for j in range(1, NCH - 1):
            emit_main_chunk(1, j * NCHUNK)
        # final chunk split in half so its serial epilogue pipelines (halves
        # keep every matmul within one 2KB psum bank)
        emit_main_chunk(1, (NCH - 1) * NCHUNK, NCHUNK // 2, tail=True)
        emit_main_chunk(1, (NCH - 1) * NCHUNK + NCHUNK // 2, NCHUNK // 2, tail=True)


@functools.lru_cache(maxsize=1)
def _build():
    nc = bacc.Bacc("TRN2", target_bir_lowering=False, debug=False)
    x2 = nc.dram_tensor("x2", [BPC, C, N], F32, kind="ExternalInput")
    x2r = nc.dram_tensor("x2r", [BPC, C, N], F32R, kind="ExternalInput")
    wallT = nc.dram_tensor("wallT", [C, 96], F32, kind="ExternalInput")
    woT = nc.dram_tensor("woT", [CG, C], F32, kind="ExternalInput")
    ident = nc.dram_tensor("ident", [32, 32], F32, kind="ExternalInput")
    y2 = nc.dram_tensor("y2", [BPC, C, N], F32, kind="ExternalOutput")
    with tile.TileContext(nc) as tc:
        _emit(nc, tc, x2, x2r, wallT, woT, ident, y2)
    nc.compile()
    return nc


def _make_in_maps(x, w_theta, w_phi, w_g, w_o, gamma):
    xf = np.ascontiguousarray(x.reshape(B, C, N), dtype=np.float32)
    wall = np.zeros((96, C), dtype=np.float32)
    wall[0:DH] = w_theta
    wall[32:32 + DH] = w_phi
    wall[64:64 + CG] = w_g
    wallT = np.ascontiguousarray(wall.T, dtype=np.float32)
    woT = np.ascontiguousarray((np.float32(gamma) * w_o).T, dtype=np.float32)
    ident = np.eye(32, dtype=np.float32)
    in_maps = []
    for c in range(NCORES):
        xc = np.ascontiguousarray(xf[c * BPC:(c + 1) * BPC])
        in_maps.append({
            "x2": xc,
            "x2r": xc,
            "wallT": wallT,
            "woT": woT,
            "ident": ident,
        })
    return in_maps


def kernel(x, w_theta, w_phi, w_g, w_o, gamma):
    nc = _build()
    in_maps = _make_in_maps(x, w_theta, w_phi, w_g, w_o, gamma)
    res = run_bass_kernel_spmd(nc, in_maps, core_ids=list(range(NCORES)))
    out = np.concatenate([res.results[c]["y2"] for c in range(NCORES)], axis=0)
    return np.ascontiguousarray(out.reshape(B, C, H, W), dtype=np.float32)


# revision 56
# speedup vs baseline: 1.0099x; 1.0099x over previous
"""SAGAN-style attention block (B=16, C=64, 64x64) on 8 TRN2 NeuronCores.

Data-parallel over batch: each core processes 2 batch elements end-to-end.

Per-batch pipeline (channel-major [C, N] layout, N = 4096 queries,
M = 1024 pooled keys):
  1. pre = [w_theta; w_phi; w_g] @ x -> [96, N] (one fused 1x1 conv matmul;
     rows padded so theta/phi/g land on 32-aligned partition bases)
  2. 2x2 maxpool of the phi/g rows over the 64x64 grid -> phi [8, M], g [32, M]
  3. scoresT[m, n] = phi^T theta computed directly in [M, N] orientation
     (lhsT = phi tile, rhs = theta chunk; contraction K = 8)
  4. E = exp(scoresT)  (no max subtraction: |s| <~ 6, exp is safe in fp32);
     exp granularity [128, 1024] (two m-tiles per psum tile) to amortize the
     per-ACTIVATE overhead — the scalar engine is the kernel's bottleneck
  5. O = [g; ones]^T-augmented PV matmul: lhsT = [gT | 1] [m, 33] so row 32
     accumulates the softmax denominator for free
  6. out = (gamma*w_o) @ O[0:32] * recip(denom) + x, denominator applied
     per-column via a gpsimd partition-broadcast

Schedule: both batches' x loads are issued up front; batch 0's pre-conv psum
copies run on the otherwise-idle scalar engine; batch 1's whole preamble is
interleaved into batch 0's main loop; batch 1's first score pairs are emitted
ahead of batch 0's final PV phase so the activation engine never drains at
the batch boundary; the final chunk is split in half to shorten the serial
epilogue. All matmuls run as float32r (1 cycle/column at N>=256, ~1.6e-4
elementwise rounding), with fp32 PSUM accumulation; end-to-end relative
error vs the fp32 reference is ~1.4e-5.
"""

import functools
import sys

import numpy as np

sys.path.insert(0, "/opt/trn_rl_repo")

import concourse.bacc as bacc
import concourse.mybir as mybir
import concourse.tile as tile
from concourse.bass_utils import run_bass_kernel_spmd

B, C, H, W = 16, 64, 64, 64
N = H * W            # 4096 queries
M = N // 4           # 1024 keys after 2x2 maxpool
NCORES = 8
BPC = B // NCORES    # batches per core
DH = 8               # theta/phi channels (C // HEADS)
CG = 32              # g channels (C // 2)
NCHUNK = 512
NCH = N // NCHUNK    # 8 chunks per batch
MT = M // 128        # 8 m-tiles of 128

F32 = mybir.dt.float32
F32R = mybir.dt.float32r
EXP = mybir.ActivationFunctionType.Exp


def _emit(nc, tc, x2, x2r, wallT, woT, ident, y2):
    with (
        tc.tile_pool(name="const", bufs=1) as pconst,
        tc.tile_pool(name="big", bufs=2) as pbig,
        tc.tile_pool(name="scr", bufs=1) as pscr,
        tc.tile_pool(name="ep", bufs=8) as pe_pool,
        tc.tile_pool(name="sm", bufs=3) as psm,
        tc.tile_pool(name="xr", bufs=6) as pxr,
        tc.tile_pool(name="psc", bufs=3, space="PSUM") as psc,
        tc.tile_pool(name="ppv", bufs=1, space="PSUM") as ppv,
        tc.tile_pool(name="pcv", bufs=1, space="PSUM") as pcv,
    ):
        # --- replicated weights: DMA in fp32, round once to fp32r ---
        wallT_s = pconst.tile([C, 96], F32)
        woT_s = pconst.tile([CG, C], F32)
        ident_s = pconst.tile([32, 32], F32)
        wallT_r = pconst.tile([C, 96], F32R)
        woT_r = pconst.tile([CG, C], F32R)
        ident_r = pconst.tile([32, 32], F32R)
        ones_c = pconst.tile([128, MT], F32)
        ones_r = pconst.tile([1, 1], F32R)

        def emit_weights_head():
            # wallT gates the pre-conv critical path; gpsimd SWDGE keeps it
            # off the HWDGE queue that the xrc loads need
            nc.gpsimd.dma_start(wallT_s[:], wallT.ap())
            nc.vector.tensor_copy(wallT_r[:], wallT_s[:])

        def emit_weights_rest():
            nc.gpsimd.dma_start(woT_s[:], woT.ap())
            nc.gpsimd.dma_start(ident_s[:], ident.ap())
            nc.vector.tensor_copy(woT_r[:], woT_s[:])
            nc.vector.tensor_copy(ident_r[:], ident_s[:])
            nc.gpsimd.memset(ones_c[:], 1.0)
            nc.vector.tensor_copy(ones_r[:], ones_c[0:1, 0:1])

        st = [{} for _ in range(BPC)]

        # All x loads for the fused conv are issued first: the cost of a DMA
        # queue slot is high and these gate the whole pre pipeline.
        XRC_PIECES = [(0, 1024), (1024, 1024), (2048, 1024), (3072, 1024)]

        def emit_xrc(b):
            st[b]["xrc"] = []
            for off, wdt in XRC_PIECES:
                xrc = pxr.tile([C, wdt], F32R, tag="xrc", name="xrc")
                nc.sync.dma_start(xrc[:], x2r.ap()[b, :, off:off + wdt])
                st[b]["xrc"].append((off, xrc))

        def emit_alloc(b):
            s_ = st[b]
            # fused conv output, rows padded for 32-aligned partition bases:
            # rows 0:8 theta, 32:40 phi, 64:96 g
            s_["pre"] = pbig.tile([96, N], F32R, tag="pre", name="pre_s")
            s_["t1p"] = pscr.tile([DH, 64, 32], F32, tag="t1phi", name="t1_phi")
            s_["t1g"] = pscr.tile([CG, 64, 32], F32, tag="t1g", name="t1_g")
            s_["phi"] = pbig.tile([DH, 32, 32], F32R, tag="phir", name="phi_r")
            s_["g"] = pbig.tile([CG, 32, 32], F32R, tag="gr", name="g_r")
            s_["gTa"] = pbig.tile([128, MT, 33], F32R, tag="gTa", name="gTa")
            nc.vector.tensor_copy(s_["gTa"][:, :, 32], ones_c[:])

        def emit_pre(b, lo, hi):
            s_ = st[b]
            for j in range(lo, hi):
                # ping-pong the pv/cv pools: during preambles they are idle
                # and the b1 pieces slot between main-loop uses with slack
                pool_j = ppv if j % 2 == 0 else pcv
                pp = pool_j.tile([96, NCHUNK], F32,
                                 tag="pv" if j % 2 == 0 else "cv")
                n0 = j * NCHUNK
                off, xrc = next(
                    (o, t) for o, t in s_["xrc"]
                    if o <= n0 < o + t.shape[1]
                )
                nc.tensor.matmul(
                    pp[:], wallT_r[:], xrc[:, n0 - off:n0 - off + NCHUNK],
                    start=True, stop=True,
                )
                dst = s_["pre"][:, j * NCHUNK:(j + 1) * NCHUNK]
                if b == 0:
                    # ACT is idle before the first exp and Copy shares every
                    # activation table with Exp (no table reload)
                    nc.scalar.copy(dst, pp[:])
                else:
                    # batch 1's copies overlap batch 0's exp-saturated window
                    nc.vector.tensor_copy(dst, pp[:])

        # 2x2 maxpool, split spatially so downstream work can start as soon
        # as the pre chunks feeding each piece are in. qn = number of
        # quarters (16 spatial rows each), qt = starting quarter.
        def emit_phi_piece(b, qt, qn):
            s_ = st[b]
            pool_in = s_["pre"][32:40, :].bitcast(F32).rearrange(
                "c (h w2 two) -> c h w2 two", two=2, w2=32
            )
            hs = slice(16 * qt, 16 * (qt + qn))
            nc.vector.tensor_max(
                s_["t1p"][:, hs, :], pool_in[:, hs, :, 0], pool_in[:, hs, :, 1]
            )
            t1v = s_["t1p"][:].rearrange("c (h2 two) w2 -> c h2 two w2", two=2)
            h2 = slice(8 * qt, 8 * (qt + qn))
            nc.vector.tensor_max(
                s_["phi"][:, h2, :], t1v[:, h2, 0, :], t1v[:, h2, 1, :]
            )

        def emit_g_half(b, hf):
            s_ = st[b]
            pool_in = s_["pre"][64:96, :].bitcast(F32).rearrange(
                "c (h w2 two) -> c h w2 two", two=2, w2=32
            )
            hs = slice(32 * hf, 32 * hf + 32)
            nc.vector.tensor_max(
                s_["t1g"][:, hs, :], pool_in[:, hs, :, 0], pool_in[:, hs, :, 1]
            )
            t1v = s_["t1g"][:].rearrange("c (h2 two) w2 -> c h2 two w2", two=2)
            h2 = slice(16 * hf, 16 * hf + 16)
            nc.vector.tensor_max(
                s_["g"][:, h2, :], t1v[:, h2, 0, :], t1v[:, h2, 1, :]
            )
            # transposes for this half's 4 m-tiles into the augmented lhsT
            g_flat = s_["g"][:].rearrange("c h w -> c (h w)")
            trp = pcv.tile([128, 128], F32R, tag="cv")
            for q in range(4):
                mi = 4 * hf + q
                nc.tensor.transpose(
                    trp[:, q * 32:(q + 1) * 32],
                    g_flat[:, mi * 128:(mi + 1) * 128],
                    ident_r[:],
                )
            nc.vector.tensor_copy(
                s_["gTa"][:, 4 * hf:4 * hf + 4, 0:32],
                trp[:].rearrange("p (four c) -> p four c", c=32),
            )

        def emit_xb(b):
            st[b]["xb"] = pbig.tile([C, N], F32, tag="xb", name="xb")
            nc.sync.dma_start(st[b]["xb"][:], x2.ap()[b])

        M_GROUPS = [(0, 1), (2, 3), (4, 5), (6, 7)]

        def emit_sc(b, n0, nw, groups):
            s_ = st[b]
            phi_r = s_["phi"][:].rearrange("c h w -> c (h w)")
            th = s_["pre"][0:8, n0:n0 + nw]  # theta chunk, f32r
            es = []
            for grp in groups:  # up to 3 m-tiles share one psum tile / exp
                scp_t = psc.tile([128, len(grp) * nw], F32, tag="sc")
                for h_, mi in enumerate(grp):
                    nc.tensor.matmul(
                        scp_t[:, h_ * nw:(h_ + 1) * nw],
                        phi_r[:, mi * 128:(mi + 1) * 128],
                        th,
                        start=True, stop=True,
                    )
                e_t = pe_pool.tile([128, len(grp) * nw], F32R, tag="E")
                nc.scalar.activation(e_t[:], scp_t[:], EXP)
                es.append(e_t)
            return es

        def emit_main_chunk(b, n0, nw=NCHUNK, tail=False, es=None):
            s_ = st[b]
            gTa = s_["gTa"]
            if es is None:
                es = emit_sc(b, n0, nw, M_GROUPS)

            pv = ppv.tile([33, nw], F32, tag="pv")
            for gi, grp in enumerate(M_GROUPS):
                for h_, mi in enumerate(grp):
                    nc.tensor.matmul(
                        pv[:],
                        gTa[:, mi, :],
                        es[gi][:, h_ * nw:(h_ + 1) * nw],
                        start=(mi == 0), stop=(mi == MT - 1),
                    )

            o_sb = psm.tile([CG, nw], F32R, tag="osb")
            nc.vector.tensor_copy(o_sb[:], pv[0:CG, :])
            cv = pcv.tile([C, nw], F32, tag="cv")
            nc.tensor.matmul(cv[:], woT_r[:], o_sb[:], start=True, stop=True)
            recip = psm.tile([1, nw], F32, tag="rc")
            nc.vector.reciprocal(recip[:], pv[CG:CG + 1, :])
            rb = psm.tile([C, nw], F32, tag="rb")
            nc.gpsimd.partition_broadcast(rb[:], recip[:])
            prod = psm.tile([C, nw], F32, tag="prod")
            nc.vector.tensor_mul(prod[:], cv[:], rb[:])
            out_sb = psm.tile([C, nw], F32, tag="out")
            if tail:
                # the kernel tail is this serial chain; DVE is faster than
                # gpsimd for the final adds
                nc.vector.tensor_add(out_sb[:], prod[:], s_["xb"][:, n0:n0 + nw])
            else:
                nc.gpsimd.tensor_add(out_sb[:], prod[:], s_["xb"][:, n0:n0 + nw])
            nc.sync.dma_start(y2.ap()[b, :, n0:n0 + nw], out_sb[:])

        # -------- schedule --------
        emit_weights_head()
        emit_xrc(0)
        emit_xrc(1)
        emit_weights_rest()
        emit_alloc(0)
        emit_pre(0, 0, 4)
        emit_phi_piece(0, 0, 2)
        emit_pre(0, 4, 8)
        emit_phi_piece(0, 2, 2)
        emit_g_half(0, 0)
        emit_g_half(0, 1)
        emit_xb(0)
        emit_alloc(1)
        # batch 1's preamble pieces ride inside batch 0's main loop: by the
        # time each piece is reached its DMA input has long landed, so the
        # PE/DVE never head-of-line block on it
        b1_pieces = {
            0: [lambda: emit_pre(1, 0, 2)],
            1: [lambda: emit_pre(1, 2, 3)],
            2: [lambda: emit_pre(1, 3, 4), lambda: emit_phi_piece(1, 0, 2)],
            3: [lambda: emit_pre(1, 4, 6), lambda: emit_g_half(1, 0)],
            4: [lambda: emit_pre(1, 6, 7)],
            5: [lambda: emit_pre(1, 7, 8), lambda: emit_phi_piece(1, 2, 2),
                lambda: emit_g_half(1, 1), lambda: emit_xb(1)],
        }
        for j in range(NCH - 1):
            emit_main_chunk(0, j * NCHUNK)
            for fn in b1_pieces.get(j, []):
                fn()
        # handoff: batch 1's first score pairs go ahead of batch 0's final
        # PV phase so the activation engine never drains at the boundary
        es_b1c0 = emit_sc(1, 0, NCHUNK, M_GROUPS[:2])
        emit_main_chunk(0, (NCH - 1) * NCHUNK)
        es_b1c0 += emit_sc(1, 0, NCHUNK, M_GROUPS[2:])
        emit_main_chunk(1, 0, es=es_b1c0)
        for j in range(1, NCH - 1):
            emit_main_chunk(1, j * NCHUNK)
        # final chunk split in half so its serial epilogue pipelines (halves
        # keep every matmul within one 2KB psum bank)
        emit_main_chunk(1, (NCH - 1) * NCHUNK, NCHUNK // 2, tail=True)
        emit_main_chunk(1, (NCH - 1) * NCHUNK + NCHUNK // 2, NCHUNK // 2, tail=True)


@functools.lru_cache(maxsize=1)
def _build():
    nc = bacc.Bacc("TRN2", target_bir_lowering=False, debug=False)
    x2 = nc.dram_tensor("x2", [BPC, C, N], F32, kind="ExternalInput")
    x2r = nc.dram_tensor("x2r", [BPC, C, N], F32R, kind="ExternalInput")
    wallT = nc.dram_tensor("wallT", [C, 96], F32, kind="ExternalInput")
    woT = nc.dram_tensor("woT", [CG, C], F32, kind="ExternalInput")
    ident = nc.dram_tensor("ident", [32, 32], F32, kind="ExternalInput")
    y2 = nc.dram_tensor("y2", [BPC, C, N], F32, kind="ExternalOutput")
    with tile.TileContext(nc) as tc:
        _emit(nc, tc, x2, x2r, wallT, woT, ident, y2)
    nc.compile()
    return nc


def _make_in_maps(x, w_theta, w_phi, w_g, w_o, gamma):
    xf = np.ascontiguousarray(x.reshape(B, C, N), dtype=np.float32)
    wall = np.zeros((96, C), dtype=np.float32)
    wall[0:DH] = w_theta
    wall[32:32 + DH] = w_phi
    wall[64:64 + CG] = w_g
    wallT = np.ascontiguousarray(wall.T, dtype=np.float32)
    woT = np.ascontiguousarray((np.float32(gamma) * w_o).T, dtype=np.float32)
    ident = np.eye(32, dtype=np.float32)
    in_maps = []
    for c in range(NCORES):
        xc = np.ascontiguousarray(xf[c * BPC:(c + 1) * BPC])
        in_maps.append({
            "x2": xc,
            "x2r": xc,
            "wallT": wallT,
            "woT": woT,
            "ident": ident,
        })
    return in_maps


def kernel(x, w_theta, w_phi, w_g, w_o, gamma):
    nc = _build()
    in_maps = _make_in_maps(x, w_theta, w_phi, w_g, w_o, gamma)
    res = run_bass_kernel_spmd(nc, in_maps, core_ids=list(range(NCORES)))
    out = np.concatenate([res.results[c]["y2"] for c in range(NCORES)], axis=0)
    return np.ascontiguousarray(out.reshape(B, C, H, W), dtype=np.float32)


# revision 62
# speedup vs baseline: 1.0129x; 1.0030x over previous
"""SAGAN-style attention block (B=16, C=64, 64x64) on 8 TRN2 NeuronCores.

Data-parallel over batch: each core processes 2 batch elements end-to-end.

Per-batch pipeline (channel-major [C, N] layout, N = 4096 queries,
M = 1024 pooled keys):
  1. pre = [w_theta; w_phi; w_g] @ x -> [96, N] (one fused 1x1 conv matmul;
     rows padded so theta/phi/g land on 32-aligned partition bases)
  2. 2x2 maxpool of the phi/g rows over the 64x64 grid -> phi [8, M], g [32, M]
  3. scoresT[m, n] = phi^T theta computed directly in [M, N] orientation
     (lhsT = phi tile, rhs = theta chunk; contraction K = 8)
  4. E = exp(scoresT)  (no max subtraction: |s| <~ 6, exp is safe in fp32);
     exp granularity [128, 1024] (two m-tiles per psum tile) to amortize the
     per-ACTIVATE overhead — the scalar engine is the kernel's bottleneck
  5. O = [g; ones]^T-augmented PV matmul: lhsT = [gT | 1] [m, 33] so row 32
     accumulates the softmax denominator for free
  6. out = (gamma*w_o) @ O[0:32] * recip(denom) + x, denominator applied
     per-column via a gpsimd partition-broadcast

Schedule: both batches' x loads are issued up front; batch 0's pre-conv psum
copies run on the otherwise-idle scalar engine; batch 1's whole preamble is
interleaved into batch 0's main loop; batch 1's first score pairs are emitted
ahead of batch 0's final PV phase so the activation engine never drains at
the batch boundary; the final chunk is split in half to shorten the serial
epilogue. All matmuls run as float32r (1 cycle/column at N>=256, ~1.6e-4
elementwise rounding), with fp32 PSUM accumulation; end-to-end relative
error vs the fp32 reference is ~1.4e-5.
"""

import functools
import sys

import numpy as np

sys.path.insert(0, "/opt/trn_rl_repo")

import concourse.bacc as bacc
import concourse.mybir as mybir
import concourse.tile as tile
from concourse.bass_utils import run_bass_kernel_spmd

B, C, H, W = 16, 64, 64, 64
N = H * W            # 4096 queries
M = N // 4           # 1024 keys after 2x2 maxpool
NCORES = 8
BPC = B // NCORES    # batches per core
DH = 8               # theta/phi channels (C // HEADS)
CG = 32              # g channels (C // 2)
NCHUNK = 512
NCH = N // NCHUNK    # 8 chunks per batch
MT = M // 128        # 8 m-tiles of 128

F32 = mybir.dt.float32
F32R = mybir.dt.float32r
EXP = mybir.ActivationFunctionType.Exp


def _emit(nc, tc, x2, x2r, wallT, woT, ident, y2):
    with (
        tc.tile_pool(name="const", bufs=1) as pconst,
        tc.tile_pool(name="big", bufs=2) as pbig,
        tc.tile_pool(name="scr", bufs=1) as pscr,
        tc.tile_pool(name="ep", bufs=8) as pe_pool,
        tc.tile_pool(name="sm", bufs=3) as psm,
        tc.tile_pool(name="xr", bufs=6) as pxr,
        tc.tile_pool(name="psc", bufs=3, space="PSUM") as psc,
        tc.tile_pool(name="ppv", bufs=1, space="PSUM") as ppv,
        tc.tile_pool(name="pcv", bufs=1, space="PSUM") as pcv,
    ):
        # --- replicated weights: DMA in fp32, round once to fp32r ---
        wallT_s = pconst.tile([C, 96], F32)
        woT_s = pconst.tile([CG, C], F32)
        ident_s = pconst.tile([32, 32], F32)
        wallT_r = pconst.tile([C, 96], F32R)
        woT_r = pconst.tile([CG, C], F32R)
        ident_r = pconst.tile([32, 32], F32R)
        ones_c = pconst.tile([128, MT], F32)
        ones_r = pconst.tile([1, 1], F32R)

        def emit_weights_head():
            # wallT gates the pre-conv critical path; gpsimd SWDGE keeps it
            # off the HWDGE queue that the xrc loads need
            nc.gpsimd.dma_start(wallT_s[:], wallT.ap())
            nc.vector.tensor_copy(wallT_r[:], wallT_s[:])

        def emit_weights_rest():
            nc.gpsimd.dma_start(woT_s[:], woT.ap())
            nc.gpsimd.dma_start(ident_s[:], ident.ap())
            nc.vector.tensor_copy(woT_r[:], woT_s[:])
            nc.vector.tensor_copy(ident_r[:], ident_s[:])
            nc.gpsimd.memset(ones_c[:], 1.0)
            nc.vector.tensor_copy(ones_r[:], ones_c[0:1, 0:1])

        st = [{} for _ in range(BPC)]

        # All x loads for the fused conv are issued first: the cost of a DMA
        # queue slot is high and these gate the whole pre pipeline.
        XRC_PIECES = [(0, 1024), (1024, 1024), (2048, 1024), (3072, 1024)]

        def emit_xrc(b):
            st[b]["xrc"] = []
            for off, wdt in XRC_PIECES:
                xrc = pxr.tile([C, wdt], F32R, tag="xrc", name="xrc")
                nc.sync.dma_start(xrc[:], x2r.ap()[b, :, off:off + wdt])
                st[b]["xrc"].append((off, xrc))

        def emit_alloc(b):
            s_ = st[b]
            # fused conv output, rows padded for 32-aligned partition bases:
            # rows 0:8 theta, 32:40 phi, 64:96 g
            s_["pre"] = pbig.tile([96, N], F32R, tag="pre", name="pre_s")
            s_["t1p"] = pscr.tile([DH, 64, 32], F32, tag="t1phi", name="t1_phi")
            s_["t1g"] = pscr.tile([CG, 64, 32], F32, tag="t1g", name="t1_g")
            s_["phi"] = pbig.tile([DH, 32, 32], F32R, tag="phir", name="phi_r")
            s_["g"] = pbig.tile([CG, 32, 32], F32R, tag="gr", name="g_r")
            s_["gTa"] = pbig.tile([128, MT, 33], F32R, tag="gTa", name="gTa")
            nc.vector.tensor_copy(s_["gTa"][:, :, 32], ones_c[:])

        def emit_pre(b, lo, hi):
            s_ = st[b]
            for j in range(lo, hi):
                # ping-pong the pv/cv pools: during preambles they are idle
                # and the b1 pieces slot between main-loop uses with slack
                pool_j = ppv if j % 2 == 0 else pcv
                pp = pool_j.tile([96, NCHUNK], F32,
                                 tag="pv" if j % 2 == 0 else "cv")
                n0 = j * NCHUNK
                off, xrc = next(
                    (o, t) for o, t in s_["xrc"]
                    if o <= n0 < o + t.shape[1]
                )
                nc.tensor.matmul(
                    pp[:], wallT_r[:], xrc[:, n0 - off:n0 - off + NCHUNK],
                    start=True, stop=True,
                )
                dst = s_["pre"][:, j * NCHUNK:(j + 1) * NCHUNK]
                if b == 0:
                    # ACT is idle before the first exp and Copy shares every
                    # activation table with Exp (no table reload)
                    nc.scalar.copy(dst, pp[:])
                else:
                    # batch 1's copies overlap batch 0's exp-saturated window
                    nc.vector.tensor_copy(dst, pp[:])

        # 2x2 maxpool, split spatially so downstream work can start as soon
        # as the pre chunks feeding each piece are in. qn = number of
        # quarters (16 spatial rows each), qt = starting quarter.
        def emit_phi_piece(b, qt, qn):
            s_ = st[b]
            pool_in = s_["pre"][32:40, :].bitcast(F32).rearrange(
                "c (h w2 two) -> c h w2 two", two=2, w2=32
            )
            hs = slice(16 * qt, 16 * (qt + qn))
            nc.vector.tensor_max(
                s_["t1p"][:, hs, :], pool_in[:, hs, :, 0], pool_in[:, hs, :, 1]
            )
            t1v = s_["t1p"][:].rearrange("c (h2 two) w2 -> c h2 two w2", two=2)
            h2 = slice(8 * qt, 8 * (qt + qn))
            nc.vector.tensor_max(
                s_["phi"][:, h2, :], t1v[:, h2, 0, :], t1v[:, h2, 1, :]
            )

        def emit_g_half(b, hf):
            s_ = st[b]
            pool_in = s_["pre"][64:96, :].bitcast(F32).rearrange(
                "c (h w2 two) -> c h w2 two", two=2, w2=32
            )
            hs = slice(32 * hf, 32 * hf + 32)
            nc.vector.tensor_max(
                s_["t1g"][:, hs, :], pool_in[:, hs, :, 0], pool_in[:, hs, :, 1]
            )
            t1v = s_["t1g"][:].rearrange("c (h2 two) w2 -> c h2 two w2", two=2)
            h2 = slice(16 * hf, 16 * hf + 16)
            nc.vector.tensor_max(
                s_["g"][:, h2, :], t1v[:, h2, 0, :], t1v[:, h2, 1, :]
            )
            # transposes for this half's 4 m-tiles into the augmented lhsT
            g_flat = s_["g"][:].rearrange("c h w -> c (h w)")
            trp = pcv.tile([128, 128], F32R, tag="cv")
            for q in range(4):
                mi = 4 * hf + q
                nc.tensor.transpose(
                    trp[:, q * 32:(q + 1) * 32],
                    g_flat[:, mi * 128:(mi + 1) * 128],
                    ident_r[:],
                )
            nc.vector.tensor_copy(
                s_["gTa"][:, 4 * hf:4 * hf + 4, 0:32],
                trp[:].rearrange("p (four c) -> p four c", c=32),
            )

        def emit_xb(b):
            st[b]["xb"] = pbig.tile([C, N], F32, tag="xb", name="xb")
            nc.sync.dma_start(st[b]["xb"][:], x2.ap()[b])

        M_GROUPS = [(0, 1), (2, 3), (4, 5), (6, 7)]

        def emit_sc(b, n0, nw, groups):
            s_ = st[b]
            phi_r = s_["phi"][:].rearrange("c h w -> c (h w)")
            th = s_["pre"][0:8, n0:n0 + nw]  # theta chunk, f32r
            es = []
            for grp in groups:  # up to 3 m-tiles share one psum tile / exp
                scp_t = psc.tile([128, len(grp) * nw], F32, tag="sc")
                for h_, mi in enumerate(grp):
                    nc.tensor.matmul(
                        scp_t[:, h_ * nw:(h_ + 1) * nw],
                        phi_r[:, mi * 128:(mi + 1) * 128],
                        th,
                        start=True, stop=True,
                    )
                e_t = pe_pool.tile([128, len(grp) * nw], F32R, tag="E")
                nc.scalar.activation(e_t[:], scp_t[:], EXP)
                es.append(e_t)
            return es

        def emit_main_chunk(b, n0, nw=NCHUNK, tail=False, es=None):
            s_ = st[b]
            gTa = s_["gTa"]
            if es is None:
                es = emit_sc(b, n0, nw, M_GROUPS)

            pv = ppv.tile([33, nw], F32, tag="pv")
            for gi, grp in enumerate(M_GROUPS):
                for h_, mi in enumerate(grp):
                    nc.tensor.matmul(
                        pv[:],
                        gTa[:, mi, :],
                        es[gi][:, h_ * nw:(h_ + 1) * nw],
                        start=(mi == 0), stop=(mi == MT - 1),
                    )

            o_sb = psm.tile([CG, nw], F32R, tag="osb")
            nc.vector.tensor_copy(o_sb[:], pv[0:CG, :])
            cv = pcv.tile([C, nw], F32, tag="cv")
            nc.tensor.matmul(cv[:], woT_r[:], o_sb[:], start=True, stop=True)
            recip = psm.tile([1, nw], F32, tag="rc")
            nc.vector.reciprocal(recip[:], pv[CG:CG + 1, :])
            rb = psm.tile([C, nw], F32, tag="rb")
            nc.gpsimd.partition_broadcast(rb[:], recip[:])
            prod = psm.tile([C, nw], F32, tag="prod")
            nc.vector.tensor_mul(prod[:], cv[:], rb[:])
            out_sb = psm.tile([C, nw], F32, tag="out")
            if tail:
                # the kernel tail is this serial chain; DVE is faster than
                # gpsimd for the final adds
                nc.vector.tensor_add(out_sb[:], prod[:], s_["xb"][:, n0:n0 + nw])
            else:
                nc.gpsimd.tensor_add(out_sb[:], prod[:], s_["xb"][:, n0:n0 + nw])
            nc.sync.dma_start(y2.ap()[b, :, n0:n0 + nw], out_sb[:])

        # -------- schedule --------
        emit_weights_head()
        emit_xrc(0)
        emit_xrc(1)
        emit_weights_rest()
        emit_alloc(0)
        emit_pre(0, 0, 4)
        emit_phi_piece(0, 0, 2)
        emit_pre(0, 4, 8)
        emit_phi_piece(0, 2, 2)
        emit_g_half(0, 0)
        emit_g_half(0, 1)
        emit_xb(0)
        emit_alloc(1)
        # batch 1's preamble pieces ride inside batch 0's main loop: by the
        # time each piece is reached its DMA input has long landed, so the
        # PE/DVE never head-of-line block on it
        b1_pieces = {
            0: [lambda: emit_pre(1, 0, 2)],
            1: [lambda: emit_pre(1, 2, 3)],
            2: [lambda: emit_pre(1, 3, 4), lambda: emit_phi_piece(1, 0, 2)],
            3: [lambda: emit_pre(1, 4, 6), lambda: emit_g_half(1, 0)],
            4: [lambda: emit_pre(1, 6, 7)],
            5: [lambda: emit_pre(1, 7, 8), lambda: emit_phi_piece(1, 2, 2),
                lambda: emit_g_half(1, 1), lambda: emit_xb(1)],
        }
        # Main stream with one-PAIR score lookahead: each chunk's first
        # score pair is emitted ahead of the previous chunk's PV phase, so a
        # PV waiting on late exps never blocks the next scores feeding the
        # activation engine, while keeping psum slot pressure low. The final
        # chunk is split in half so its serial epilogue pipelines (halves
        # keep every matmul within one 2KB psum bank).
        units = [(0, j * NCHUNK, NCHUNK, False) for j in range(NCH)]
        units += [(1, j * NCHUNK, NCHUNK, False) for j in range(NCH - 1)]
        units += [(1, (NCH - 1) * NCHUNK, NCHUNK // 2, True),
                  (1, (NCH - 1) * NCHUNK + NCHUNK // 2, NCHUNK // 2, True)]
        carry = None
        for i, (b, n0, nw, tl) in enumerate(units):
            if carry is None:
                es = emit_sc(b, n0, nw, M_GROUPS)
            else:
                es = carry + emit_sc(b, n0, nw, M_GROUPS[2:])
            if i + 1 < len(units):
                nb, nn0, nnw, _ = units[i + 1]
                carry = emit_sc(nb, nn0, nnw, M_GROUPS[:2])
            else:
                carry = None
            emit_main_chunk(b, n0, nw, tail=tl, es=es)
            if b == 0:
                for fn in b1_pieces.get(n0 // NCHUNK, []):
                    fn()


@functools.lru_cache(maxsize=1)
def _build():
    nc = bacc.Bacc("TRN2", target_bir_lowering=False, debug=False)
    x2 = nc.dram_tensor("x2", [BPC, C, N], F32, kind="ExternalInput")
    x2r = nc.dram_tensor("x2r", [BPC, C, N], F32R, kind="ExternalInput")
    wallT = nc.dram_tensor("wallT", [C, 96], F32, kind="ExternalInput")
    woT = nc.dram_tensor("woT", [CG, C], F32, kind="ExternalInput")
    ident = nc.dram_tensor("ident", [32, 32], F32, kind="ExternalInput")
    y2 = nc.dram_tensor("y2", [BPC, C, N], F32, kind="ExternalOutput")
    with tile.TileContext(nc) as tc:
        _emit(nc, tc, x2, x2r, wallT, woT, ident, y2)
    nc.compile()
    return nc


def _make_in_maps(x, w_theta, w_phi, w_g, w_o, gamma):
    xf = np.ascontiguousarray(x.reshape(B, C, N), dtype=np.float32)
    wall = np.zeros((96, C), dtype=np.float32)
    wall[0:DH] = w_theta
    wall[32:32 + DH] = w_phi
    wall[64:64 + CG] = w_g
    wallT = np.ascontiguousarray(wall.T, dtype=np.float32)
    woT = np.ascontiguousarray((np.float32(gamma) * w_o).T, dtype=np.float32)
    ident = np.eye(32, dtype=np.float32)
    in_maps = []
    for c in range(NCORES):
        xc = np.ascontiguousarray(xf[c * BPC:(c + 1) * BPC])
        in_maps.append({
            "x2": xc,
            "x2r": xc,
            "wallT": wallT,
            "woT": woT,
            "ident": ident,
        })
    return in_maps


def kernel(x, w_theta, w_phi, w_g, w_o, gamma):
    nc = _build()
    in_maps = _make_in_maps(x, w_theta, w_phi, w_g, w_o, gamma)
    res = run_bass_kernel_spmd(nc, in_maps, core_ids=list(range(NCORES)))
    out = np.concatenate([res.results[c]["y2"] for c in range(NCORES)], axis=0)
    return np.ascontiguousarray(out.reshape(B, C, H, W), dtype=np.float32)
